# revision 41
# baseline (speedup 1.0000x reference)
"""Trainium2 Bass kernel for an attention block (B=16, C=512, T=2048).

reference:
  q = wq@x + bq; k = wk@x + bk; v = wv@x + bv          (conv1x1 per sample)
  attn = softmax(q^T k over s); out = v @ attn^T
  result = gamma * out + x
Sharding: data-parallel over batch across 8 NeuronCores (2 samples/core),
weights replicated.

Device algorithm:
  - host folds gamma into wv, and gamma*bv + x into the residual xg
    (softmax rows sum to 1, so the v-bias is a per-channel constant);
    bk is dropped (a per-t constant in scores cancels in softmax over s).
  - all DRAM tensors are host-swizzled into the exact SBUF layout
    ([partition, ...] with >=1KB contiguous runs per partition) so every
    DMA moves large packets; the output is de-swizzled on host.
  - q/k/scores path in fp16 (1 PE cycle/row); v/softmax-weights path in
    bf16 (range: exp(S) reaches ~e^64); PSUM accumulation always fp32.
  - phase 1 (both samples up front): v^T[s,o] tiles via
    matmul(lhsT=x[c,s], rhs=(g*wv)^T[c,o]), q/k via one M=128 matmul
    (k rows 0:64, q rows 64:128 with bias; q then DMA-shifted to
    partitions 0:64 so S^T operands share a partition range).
  - phase 2, per 512-wide t-chunk, per pair of 128-wide s-chunks
    (pipelined one pair ahead, crossing chunk/sample boundaries):
      S^T[s,t] = matmul(lhsT=k[:,s], rhs=q[:,t])  (fp16; the pair's two
                 K=64 matmuls are ROW-PACKED at rows 0:63 / 64:127 and
                 partially co-issue on the PE via auto row tiling)
      E = exp(S^T)  (ACT, PSUM -> SBUF bf16; split per half for the
                 chunk's FIRST pair only, to release the stp PSUM WAR to
                 the next S^T pair earlier)
      out0[c,t] += matmul(lhsT=v^T[s,c], rhs=E)    (bf16, 4 c-chunks)
      softmax denominator OFF the PE: E2 = E_a + E_b on GpSimd, folded
      into a running fp32 accumulator on the DVE (last pair folds
      directly on the DVE, emitting bf16), then ONE
      den = matmul(lhsT=ones128, rhs=acc_bf16) per chunk
      (partition-reduce + broadcast; this removed 7/8 of the den
      matmul columns and ~16us of PE time vs per-pair den matmuls)
    then per chunk: out0 -> SBUF (bf16 ACT copies -- half the scalar
      time of fp32, so the next chunk's exps don't queue behind them),
      r = recip_approx(den) (DVE), result = out0*r + xg (DVE, bf16 out)
      -> one contiguous DMA per chunk; host upcasts/deswizzles.
  - the LAST t-chunk runs as two 256-wide sub-steps so its finals
    overlap the prior sub-step's PE work instead of forming a serial
    DVE tail after the final matmul (128-wide quarters measurably
    inflate the fine-grained matmul/ACT stream; 256 is the sweet spot).
  - head: warm-up matmuls on a gpsimd-memset tile (gpsimd wakes ~6us,
    before any DMA lands) keep the PE ramping during the input DMA
    wait; each warm matmul has DISTINCT args or the value cache dedups
    them away.  x/wv loads ride the two hardware-DGE queues (sync,
    scalar); the gpsimd software-DGE queue (~0.7us per issue) only
    carries tensors needed late.

Perf notes (measured via NTFF traces on the axon-tunneled device):
  - PE-bound: Tensor busy ~183us of ~208us exec (95%+ of its span);
    matmul floor is 1 column/cycle at the device clock (215ns per
    N=512 bf16/fp16 matmul at 2.38GHz), clock/throttle state varies
    run to run (same code measured 207.7-212.8us; older clock states
    gave 222-272us).
  - fp8e4 DoubleRow matmuls (2x) were evaluated and REJECTED: DR needs
    both operands fp8, and e4m3 vt quantization alone produces 3.3e-2
    max rel err (gate: 2e-2) because peaked softmax rows pass single-v
    quantization errors straight through; hi-lo fp8 splits restore
    accuracy but double the contraction, exactly cancelling the 2x.
  - splitting EVERY exp in two (not just the chunk-boundary one)
    shrinks the S^T stp-WAR stalls but inflates the whole out-matmul
    stream by ~11ns/mm -- net loss.  Moving the vt PSUM->SBUF casts
    from DVE to scalar stalls the phase-1 vps bank-reuse chains.
  - S^T K=64 row-packed pairs run at ~376ns (vs 430 serial, 215
    ideal): the 64x64 4-tile variant needs mixed-s vt layouts that
    would double phase-1 cost -- parked.
"""
import numpy as np
import ml_dtypes
import concourse.bass as bass
import concourse.bacc as bacc
import concourse.tile as tile
from concourse import mybir
from concourse.bass_utils import run_bass_kernel_spmd

F32 = mybir.dt.float32
FP16 = mybir.dt.float16
BF16 = mybir.dt.bfloat16
AF = mybir.ActivationFunctionType

B, C, T, D = 16, 512, 2048, 64
NCORES = 8
BPC = B // NCORES          # samples per core
CCH = C // 128             # 4 channel chunks
TW = 512                   # t tile width (matmul free dim)
TCH = T // TW              # 4 t chunks
SCH = T // 128             # 16 s chunks
NPR = SCH // 2             # 8 s-chunk pairs

PROFILE = False            # set True before calling kernel() to capture HW time
LAST_EXEC_NS = None
_CACHE = {}


def _build():
    nc = bacc.Bacc("TRN2", target_bir_lowering=False, debug=False,
                   enable_asserts=False)
    # all tensors host-swizzled to SBUF layout (partition dim first)
    xd = nc.dram_tensor("x", [BPC, 128, SCH, CCH, 128], FP16,
                        kind="ExternalInput").ap()
    xgd = nc.dram_tensor("xg", [BPC, TCH, 128, CCH, TW], BF16,
                         kind="ExternalInput").ap()
    wkqd = nc.dram_tensor("wkq", [128, CCH, 2 * D], FP16,
                          kind="ExternalInput").ap()
    wvd = nc.dram_tensor("wv", [128, CCH, C], FP16,
                         kind="ExternalInput").ap()
    bqd = nc.dram_tensor("bq", [D, 1], F32, kind="ExternalInput").ap()
    onesd = nc.dram_tensor("ones", [128, 128], BF16, kind="ExternalInput").ap()
    outd = nc.dram_tensor("out", [BPC, TCH, 128, CCH, TW], BF16,
                          kind="ExternalOutput").ap()

    with tile.TileContext(nc) as tc:
        with tc.tile_pool(name="const", bufs=1) as constp, \
             tc.tile_pool(name="xp", bufs=1) as xp, \
             tc.tile_pool(name="vtp", bufs=1) as vtp, \
             tc.tile_pool(name="qkp", bufs=1) as qkp, \
             tc.tile_pool(name="etp", bufs=1) as etp, \
             tc.tile_pool(name="finp", bufs=1) as finp, \
             tc.tile_pool(name="ps", bufs=1, space="PSUM") as ps:

            # ---- input loads: contiguous slice DMAs over 3 queues ----
            x_big_all = [xp.tile([128, SCH, CCH, 128], FP16, name=f"x_{b}",
                                 tag=f"x{b}") for b in range(BPC)]
            wv_big = constp.tile([128, CCH, C], FP16)
            wkq_big = constp.tile([128, CCH, 2 * D], FP16)
            ones = constp.tile([128, 128], BF16)
            bq_full = constp.tile([128, 1], F32)

            # the first vt matmul needs wv (4 cc) + x0 slice 0 only; spread
            # across sync/scalar/gpsimd queues so the PE starts right after
            # the fixed NEFF preamble
            # PE p-state warm-up operand: memset FIRST on gpsimd (the
            # earliest-waking engine, ~6us) and BEFORE its dma_starts (each
            # software-DGE issue costs ~0.7us of engine time) so the warm
            # matmuls are not gated on any DMA completion
            warm = constp.tile([128, TW], BF16)
            nc.gpsimd.memset(warm[:], 0)

            def load_x(eng, b, lo, hi):
                # x is host-swizzled [p, s, c, t]: per-partition runs are
                # (hi-lo)*1KB CONTIGUOUS in DRAM (no rearrange), so the DMA
                # moves large packets
                eng.dma_start(
                    out=x_big_all[b][:, lo:hi], in_=xd[b, :, lo:hi])

            nc.sync.dma_start(out=x_big_all[0][:, 0], in_=xd[0, :, 0])
            nc.scalar.dma_start(out=wv_big[:, 2:4, :], in_=wvd[:, 2:4, :])
            # on sync (hardware DGE): the gpsimd software-DGE issue path
            # does not start until ~7us, which would gate the first real
            # vt matmul on this wv half
            nc.sync.dma_start(out=wv_big[:, 0:2, :], in_=wvd[:, 0:2, :])
            load_x(nc.sync, 0, 1, 2)
            load_x(nc.sync, 0, 2, 4)
            load_x(nc.scalar, 0, 4, 6)
            load_x(nc.sync, 0, 6, 8)
            load_x(nc.sync, 0, 10, 12)
            load_x(nc.scalar, 0, 8, 10)
            load_x(nc.sync, 0, 14, 16)
            load_x(nc.scalar, 0, 12, 14)
            nc.gpsimd.dma_start(out=wkq_big, in_=wkqd)
            nc.gpsimd.dma_start(out=ones, in_=onesd)
            nc.gpsimd.dma_start(out=bq_full[D:2 * D, :], in_=bqd)
            load_x(nc.sync, 1, 0, 4)
            load_x(nc.scalar, 1, 4, 8)
            load_x(nc.gpsimd, 1, 8, 12)
            load_x(nc.gpsimd, 1, 12, 16)
            bq_hi = bq_full[D:2 * D, :]
            wv_sb = [wv_big[:, cc, :] for cc in range(CCH)]
            wkq_sb = [wkq_big[:, cc, :] for cc in range(CCH)]

            # PE p-state warm-up: dummy matmuls on the memset tile run
            # during the initial DMA wait so the real phase-1 matmuls start
            # at full clock (the PE needs ~3us of continuous work to ramp).
            # each matmul has DISTINCT args -- identical instructions get
            # deduplicated by the value cache and the warm-up vanishes
            wps = ps.tile([128, 2, TW], F32, name="warm_ps", tag="stp")
            # 14 matmuls ~= the ~6us DMA wait for x/wv: the PE stays busy
            # (and its clock ramping) right up to the first real vt chain
            for i in range(14):
                j = 8 * (i // 2)
                nc.tensor.matmul(wps[:, i % 2, j:j + 448],
                                 warm[:, 0:128], warm[:, j:j + 448],
                                 start=True, stop=True)

            # ================= phase 1: v^T and q/k, both samples ========
            vt_all, q_all, k_all = {}, {}, {}
            et = {}

            def emit_st2(b, tc_i, pr, lo=0, w=TW):
                # the pair's two K=64 fp16 S^T matmuls co-issue (rows 0:63
                # and 64:127); one ACT exp over both PSUM banks
                tsl = slice(tc_i * TW + lo, tc_i * TW + lo + w)
                stp = ps.tile([128, 2, TW], F32,
                              name=f"st_{b}_{tc_i}_{pr}_{lo}", tag="stp")
                # FULL K=128 contraction: k/q live in BOTH partition halves
                # (host halves wq so the doubled dot product is exact), so
                # these matmuls stay in the 128x128 tiling mode like every
                # other matmul in the kernel.  K=64 row-packed pairs co-issue
                # perfectly (1024 cols in ~218ns) but the 64x128<->128x128
                # mode switch forces a PE drain that un-hides one LDWEIGHTS
                # on BOTH sides of every pair (+~110ns each) -- measured
                # net-worse than paying the serialized K=128 streams
                for h in range(2):
                    sc = 2 * pr + h
                    nc.tensor.matmul(
                        stp[:, h, 0:w],
                        k_all[b][:, sc * 128:(sc + 1) * 128],
                        q_all[b][:, tsl],
                        start=True, stop=True)
                t_et = etp.tile([128, 2, w], BF16,
                                name=f"et_{b}_{tc_i}_{pr}_{lo}", tag=f"et{pr}")
                if pr == 0 and w == TW:
                    # chunk-boundary exp split per half: the next pair's h0
                    # matmul only waits for the h0 exp to drain its stp bank.
                    # boundary-only: splitting EVERY exp measurably inflates
                    # the out-matmul stream (~+11ns/mm)
                    for h in range(2):
                        nc.scalar.activation(out=t_et[:, h, :],
                                             in_=stp[:, h, 0:w], func=AF.Exp)
                else:
                    nc.scalar.activation(out=t_et[:, :, :],
                                         in_=stp[:, :, 0:w], func=AF.Exp)
                et[(b, tc_i, pr, lo)] = t_et

            for b in range(BPC):
                x_b = x_big_all[b]

                # v^T tiles (bf16): vt[sc][s=128, o=512]
                vt_sb = []
                for sc in range(SCH):
                    vps = ps.tile([128, TW], F32, name=f"vps_{b}_{sc}",
                                  tag=f"o{sc % 2}")
                    for cc in range(CCH):
                        nc.tensor.matmul(
                            vps[:], x_b[:, sc, cc, :], wv_sb[cc][:],
                            start=(cc == 0), stop=(cc == CCH - 1))
                    t_vt = vtp.tile([128, C], BF16, name=f"vt_{b}_{sc}",
                                    tag=f"vt_{b}_{sc}")
                    nc.vector.tensor_copy(out=t_vt[:], in_=vps[:])
                    vt_sb.append(t_vt)
                vt_all[b] = vt_sb

                # q, k via one M=128 matmul; q and k each REPLICATED into
                # both partition halves of ONE tile (same SBUF columns), so
                # the two co-issued S^T matmuls of a pair read their rhs
                # streams from the same fetch lines (h0 from partitions
                # 0:64, h1 from 64:128)
                q2 = qkp.tile([128, T], FP16, name=f"q_{b}", tag=f"q{b}")
                k2 = qkp.tile([128, T], FP16, name=f"k_{b}", tag=f"k{b}")
                for tc_i in range(TCH):
                    tsl = slice(tc_i * TW, (tc_i + 1) * TW)
                    qps = ps.tile([128, TW], F32, name=f"qps_{b}_{tc_i}",
                                  tag=f"o{2 + tc_i % 2}")
                    for cc in range(CCH):
                        nc.tensor.matmul(
                            qps[:], wkq_sb[cc][:],
                            x_b[:, 4 * tc_i:4 * (tc_i + 1), cc, :],
                            start=(cc == 0), stop=(cc == CCH - 1))
                    nc.vector.tensor_copy(out=k2[0:D, tsl], in_=qps[0:D, :])
                    nc.scalar.activation(out=q2[D:2 * D, tsl],
                                         in_=qps[D:2 * D, :],
                                         func=AF.Identity, bias=bq_hi[:],
                                         scale=1.0)
                    # on gpsimd: the scalar queue must stay free for the
                    # q-bias ACTs + first exps at the phase-1/2 boundary
                    # (each dma_start costs ~0.5us of issuing-queue time)
                    nc.gpsimd.dma_start(out=q2[0:D, tsl],
                                        in_=q2[D:2 * D, tsl])
                    nc.gpsimd.dma_start(out=k2[D:2 * D, tsl],
                                        in_=k2[0:D, tsl])
                q_all[b], k_all[b] = q2, k2

                if b == 0:
                    # first S^T/exp pair warms up under sample 1's prework
                    emit_st2(0, 0, 0, 0, TW)

            # ================= phase 2: attention, all chunks ============
            # The LAST chunk is split into 128-wide sub-steps so its finals
            # (recip + mul/add on DVE) overlap the next sub-step's PE work
            # instead of forming a ~15us serial tail after the last matmul.
            QW = 256
            tasks = [(b, tc_i, 0, TW)
                     for b in range(BPC) for tc_i in range(TCH)][:-1]
            tasks += [(BPC - 1, TCH - 1, q * QW, QW) for q in range(TW // QW)]
            xg_cur = {}
            for si, (b, tc_i, lo, w) in enumerate(tasks):
                sub = w != TW
                oacc = [ps.tile([128, w], F32,
                                name=f"o_{b}_{tc_i}_{cc}_{lo}", tag=f"o{cc}")
                        for cc in range(CCH)]
                if lo == 0:
                    xg_t = finp.tile([128, CCH, TW], BF16,
                                     name=f"xg_{b}_{tc_i}", tag="xg", bufs=3)
                    nc.sync.dma_start(out=xg_t, in_=xgd[b, tc_i])
                    xg_cur[(b, tc_i)] = xg_t
                xg_t = xg_cur[(b, tc_i)]

                for pr in range(NPR):
                    # keep one S^T/exp pair in flight ahead of the consumers
                    if pr + 1 < NPR:
                        emit_st2(b, tc_i, pr + 1, lo, w)
                    elif si + 1 < len(tasks):
                        nb, ntc, nlo, nw = tasks[si + 1]
                        emit_st2(nb, ntc, 0, nlo, nw)
                    e = et.pop((b, tc_i, pr, lo))
                    # the softmax denominator is accumulated OFF the PE:
                    # pair halves are summed on gpsimd, folded into a
                    # running fp32 accumulator on the DVE, and only ONE
                    # ones-matmul per chunk (below) does the partition-
                    # reduce + broadcast: removes 7/8 of den matmul columns
                    if pr == 0:
                        acc = finp.tile([128, w], F32,
                                        name=f"acc_{b}_{tc_i}_{lo}",
                                        tag="acc" if w == TW else "accs",
                                        bufs=2)
                        nc.vector.tensor_add(acc[:], e[:, 0, :], e[:, 1, :])
                    elif pr < NPR - 1:
                        e2 = etp.tile([128, w], BF16,
                                      name=f"e2_{b}_{tc_i}_{pr}_{lo}",
                                      tag="e2", bufs=3)
                        nc.gpsimd.tensor_add(e2[:], e[:, 0, :], e[:, 1, :])
                        nc.vector.tensor_add(acc[:], acc[:], e2[:])
                    else:
                        # last pair folds straight in on the DVE (a gpsimd
                        # hop costs ~1.5us and would gate the den matmul at
                        # the chunk boundary); final add emits bf16 so the
                        # den matmul stays at 1 PE cycle/row
                        nc.vector.tensor_add(acc[:], acc[:], e[:, 0, :])
                        e2b = etp.tile([128, w], BF16,
                                       name=f"e2b_{b}_{tc_i}_{lo}",
                                       tag="e2b", bufs=2)
                        nc.vector.tensor_add(e2b[:], acc[:], e[:, 1, :])
                    for h in range(2):
                        sc = 2 * pr + h
                        esl = e[:, h, :]
                        for cc in range(CCH):
                            nc.tensor.matmul(
                                oacc[cc],
                                vt_all[b][sc][:, cc * 128:(cc + 1) * 128],
                                esl, start=(sc == 0), stop=(sc == SCH - 1))

                # finals: free the o banks fast (bf16 ACT copies -- half
                # the scalar time of fp32, so the next chunk's exps don't
                # queue behind them), then den matmul + the slow DVE
                # reciprocal + mul/add run off the PE critical path
                last = si == len(tasks) - 1
                o_srcs = []
                for cc in range(CCH):
                    if last:
                        o_srcs.append(oacc[cc])  # tail: no bank hurry
                    else:
                        t_o = finp.tile([128, w], BF16,
                                        name=f"ob_{b}_{tc_i}_{cc}_{lo}",
                                        tag=f"ob{cc}", bufs=2)
                        nc.scalar.activation(out=t_o[:], in_=oacc[cc],
                                             func=AF.Copy)
                        o_srcs.append(t_o[:])
                den = ps.tile([128, w], F32, name=f"den_{b}_{tc_i}_{lo}",
                              tag="den", bufs=2)
                nc.tensor.matmul(den[:], ones[:], e2b[:],
                                 start=True, stop=True)
                recip = finp.tile([128, w], F32, name=f"rc_{b}_{tc_i}_{lo}",
                                  tag="rc", bufs=2)
                # den is in [1, ~1e25] (top softmax term is >= 1), far from
                # the approx-recip edge cases; ~18 correct bits is plenty
                nc.vector.reciprocal_approx_fast(out=recip[:], in_=den[:])
                t_f = finp.tile([128, CCH, w], BF16, name=f"f_{b}_{tc_i}_{lo}",
                                tag="f", bufs=2)
                for cc in range(CCH):
                    nc.vector.tensor_mul(t_f[:, cc, :], o_srcs[cc], recip[:])
                    nc.vector.tensor_add(t_f[:, cc, :], t_f[:, cc, :],
                                         xg_t[:, cc, lo:lo + w])
                    if last:
                        # tail: ship each c-chunk as soon as its add lands,
                        # alternating the two hardware-DGE queues so the
                        # final drain is not serialized on one ring
                        eng = nc.sync if cc % 2 == 0 else nc.scalar
                        eng.dma_start(
                            out=outd[b, tc_i, :, cc, lo:lo + w],
                            in_=t_f[:, cc, :])
                if not last:
                    if sub:
                        nc.sync.dma_start(out=outd[b, tc_i, :, :, lo:lo + w],
                                          in_=t_f)
                    else:
                        nc.sync.dma_start(out=outd[b, tc_i], in_=t_f)
    nc.compile()
    return nc


def _get_nc():
    if "nc" not in _CACHE:
        _CACHE["nc"] = _build()
    return _CACHE["nc"]


def kernel(x, wq, bq, wk, bk, wv, bv, gamma):
    global LAST_EXEC_NS
    g = float(np.asarray(gamma).reshape(-1)[0])
    x = np.asarray(x, np.float32)
    # fold gamma into the v path; bk cancels inside softmax; the v bias
    # contributes gamma*bv per channel (softmax rows sum to 1) -> fold it
    # plus the residual into xg
    wvT = np.ascontiguousarray(
        (g * np.asarray(wv, np.float32)).T).astype(np.float16)
    wv_sw = np.ascontiguousarray(
        wvT.reshape(CCH, 128, C).transpose(1, 0, 2))          # [p, cc, o]
    # wq/bq halved (exact in fp16): the device S^T matmul contracts K=128
    # with k and q each replicated into both partition halves, computing
    # 2*(k . q/2) = k . q while staying in the uniform 128x128 tiling mode
    wkqT = np.concatenate([np.asarray(wk, np.float32).T,
                           0.5 * np.asarray(wq, np.float32).T],
                          axis=1).astype(np.float16)
    wkq_sw = np.ascontiguousarray(
        wkqT.reshape(CCH, 128, 2 * D).transpose(1, 0, 2))     # [p, cc, d]
    bq2 = 0.5 * np.asarray(bq, np.float32).reshape(D, 1)
    gbv = (g * np.asarray(bv, np.float32)).reshape(1, C, 1)
    xg = x + gbv
    ones = np.ones((128, 128), ml_dtypes.bfloat16)
    xh = x.astype(np.float16)
    # swizzle to SBUF layouts: x -> [b, p, s, cc, t128] (per-partition data
    # contiguous in DRAM -> large DMA packets), xg -> [b, ch, p, cc, t512]
    x_sw = np.ascontiguousarray(
        xh.reshape(B, CCH, 128, SCH, 128).transpose(0, 2, 3, 1, 4))
    xg_sw = np.ascontiguousarray(
        xg.reshape(B, CCH, 128, TCH, TW).transpose(0, 3, 2, 1, 4)).astype(
            ml_dtypes.bfloat16)

    in_maps = []
    for core in range(NCORES):
        sl = slice(core * BPC, (core + 1) * BPC)
        in_maps.append({
            "x": x_sw[sl], "xg": xg_sw[sl],
            "wkq": wkq_sw, "wv": wv_sw,
            "bq": bq2, "ones": ones,
        })

    nc = _get_nc()
    res = run_bass_kernel_spmd(nc, in_maps, core_ids=list(range(NCORES)),
                               trace=PROFILE)
    LAST_EXEC_NS = res.exec_time_ns
    out = np.empty((B, C, T), np.float32)
    for core in range(NCORES):
        o = np.asarray(res.results[core]["out"], np.float32)
        out[core * BPC:(core + 1) * BPC] = np.ascontiguousarray(
            o.transpose(0, 3, 2, 1, 4)).reshape(BPC, C, T)
    return out

